# revision 1
# baseline (speedup 1.0000x reference)
"""Trainium2 Bass kernel for nn_Capsule (capsule routing with dynamic routing).

reference: u = x @ W  (per-sample [512,256]@[256,512] -> [512, (32 o, 16 f)])
           b=0; 3x { c = softmax_o(b); v[o,f] = sum_i c[o,i] u[i,(o,f)];
                     v = squash(v); b[o,i] = sum_f v[o,f] u[i,(o,f)] }
           return v  [B, 32, 16]

Key algebraic restructuring (u is NEVER materialized):
  v_raw[o,f] = sum_i c[o,i] u[i,(o,f)]  =  diag-extract[ (c @ x) @ W ]
      y = c @ x      (PE: cT stationary [i,32], x natural moving)
      vfull = y @ W  (PE: yT stationary, W natural moving)
      v_raw = mask * vfull, then per-sample partition-sum via indicator matmul
  b[o,i] = sum_f v[o,f] u[i,(o,f)] = sum_h z[o,h] x[i,h]
      z[o,h] = sum_f v[o,f] W[h,(o,f)]  (PE: block-diag Vmat stationary, WT moving)
      b = z @ xT     (PE: zT stationary col-tiled, xT moving)
  softmax over o on bT [i-partition, o-free] via PE transposes of exp(b).

16 samples/core x 8 cores; per core 2 half-batches of 2 packs x 4 samples;
a pack's 4 samples run concurrently via PE column tiling tile_position=(0,32s).
"""

import numpy as np

import concourse.bass as bass
import concourse.tile as tile
from concourse import mybir
from concourse.bass_utils import run_bass_kernel_spmd

F32 = mybir.dt.float32
R32 = mybir.dt.float32r
AF = mybir.ActivationFunctionType
AX = mybir.AxisListType

B, I, H = 128, 512, 256
O, F = 32, 16
OF = O * F  # 512
NCORES = 8
S = B // NCORES      # 16 samples per core
NHB = 2              # half-batches per core
NPK = 2              # packs per half-batch
PK = 4               # samples per pack (col-tiling width)
NITER = 3
P = 128

# constant-blob layout (one DMA, per-partition element offsets)
CW = 0                  # W  [h%128, (hc 2, of 512)]
CWT = CW + 2 * OF       # WT [of%128, (m 4, h 256)]
CID = CWT + 4 * H       # identity [128, 128]
CMC = CID + P           # diag mask [128, 512]
CS4 = CMC + OF          # sample-sum indicator [128, 4]
CBM = CS4 + PK          # Vmat block masks [128, (m 4, j 32)]
CC0 = CBM + 4 * O       # uniform 1/32 [128, 32]
CIDR = CC0 + O          # identity again, viewed as float32r by device
CSTN = CIDR + P


def ap(t, dims, off=0):
    """AP over tile/handle `t`: keep partition dim, explicit free dims."""
    a = t if isinstance(t, bass.AP) else t[:]
    return bass.AP(tensor=a.tensor, offset=a.offset + off,
                   ap=[list(a.ap[0])] + [list(d) for d in dims])


def fview(a):
    """Alias a float32r AP as plain fp32 (same bytes) for transposes/DVE."""
    t = a.tensor
    if t.dtype != R32:
        return a
    t2 = bass.SBTensorHandle(name=t.name, shape=t.shape, dtype=F32,
                             base_partition=t.base_partition,
                             manual_sbuf_range=t.manual_sbuf_range,
                             manual_base_name=t.manual_base_name)
    return bass.AP(tensor=t2, offset=a.offset,
                   ap=[list(d) for d in a.ap])


def rview(a):
    """Alias an fp32 AP as float32r (same bytes, PE fast-path dtype)."""
    t = a.tensor
    t2 = bass.SBTensorHandle(name=t.name, shape=t.shape, dtype=R32,
                             base_partition=t.base_partition,
                             manual_sbuf_range=t.manual_sbuf_range,
                             manual_base_name=t.manual_base_name)
    return bass.AP(tensor=t2, offset=a.offset,
                   ap=[list(d) for d in a.ap])


def dram_ap(handle, dims, off=0):
    """AP over DRAM handle with fully explicit dims (first = partition)."""
    a = handle[:]
    return bass.AP(tensor=a.tensor, offset=a.offset + off,
                   ap=[list(d) for d in dims])


def build_program(split_waits=True):
    nc = bass.Bass("TRN2", target_bir_lowering=False)

    x_d = nc.dram_tensor("x", [S, I, H], F32, kind="ExternalInput")
    cst_d = nc.dram_tensor("cst", [P, CSTN], F32, kind="ExternalInput")
    out_d = nc.dram_tensor("out", [S, OF], F32, kind="ExternalOutput")

    with tile.TileContext(nc) as tc:
        with (
            tc.tile_pool(name="consts", bufs=1) as consts,
            tc.tile_pool(name="xpool", bufs=4) as xpool,
            tc.tile_pool(name="xtpool", bufs=4) as xtpool,
            tc.tile_pool(name="work", bufs=2) as work,
            tc.tile_pool(name="sm", bufs=2) as sm,
            tc.tile_pool(name="ps_big", bufs=1, space="PSUM") as ps_big,
            tc.tile_pool(name="ps_mid", bufs=2, space="PSUM") as ps_mid,
            tc.tile_pool(name="ps_xt", bufs=2, space="PSUM") as ps_xt,
            tc.tile_pool(name="ps_anch", bufs=1, space="PSUM") as ps_anch,
        ):
            # ---- constants: ONE DMA so downstream PE ops wait on one sem ----
            cst = consts.tile([P, CSTN], F32)
            nc.sync.dma_start(out=cst[:], in_=cst_d[:])
            w_sb = cst[:, CW:CW + 2 * OF]        # [h%128, (hc, of)]
            wt_sb = cst[:, CWT:CWT + 4 * H]      # [of%128, (m, h)]
            id_sb = cst[:, CID:CID + P]          # identity
            mc_sb = cst[:, CMC:CMC + OF]         # diag mask (p%32 == o)
            s4_sb = cst[:, CS4:CS4 + PK]         # s4[p,s] = (p//32 == s)
            bm_sb = cst[:, CBM:CBM + 4 * O]      # bm[p,(m,j)]=(j==8m+p//16)
            c0_sb = cst[:, CC0:CC0 + O]          # uniform 1/32 (iter 0)

            # PE sync anchors: every datapath instruction carries at most ONE
            # sem wait (walrus).  A 1x1 transpose reading a byte of a dirty
            # foreign-engine tensor makes PE "observe" that engine's clock so
            # later PE instructions need no cross-engine waits.
            anch = ps_anch.tile([P, F], F32)
            dirty = {}
            acol = [0]
            pending = []

            def mark(key, apv):
                dirty[key] = apv

            def pe_sync(*keys):
                pending.clear()
                for k in keys:
                    if k not in dirty:
                        continue
                    d = dirty.pop(k)
                    a = nc.tensor.transpose(
                        anch[:1, acol[0]:acol[0] + 1], d[:1, :1],
                        id_sb[:1, :1])
                    pending.append(a.ins)
                    acol[0] = (acol[0] + 1) % F

            def _chain(b):
                for a in pending:
                    bass._add_dep_helper(b.ins, a, sync=False,
                                         reason="pe-anchor order")
                return b

            def T(out, in_, ident):
                return _chain(nc.tensor.transpose(out, in_, ident))

            def MM(out, lhsT, rhs, **kw):
                return _chain(nc.tensor.matmul(out, lhsT, rhs, **kw))

            def dep(b, a):
                if a is not None:
                    bass._add_dep_helper(b.ins, a, sync=False,
                                         reason="engine-anchor order")
                return b

            mark("cst", cst)

            dscr = sm.tile([PK, PK], F32, tag="dscr")
            nc.vector.memset(dscr[:], 0.0)
            # one-time: let DVE observe the const DMA (mc/bm reads)
            dcst_a = nc.vector.tensor_copy(dscr[:1, :1], cst[:1, :1]).ins

            # ---- load x (natural [i, h]); build xT via PE transposes ----
            x_sb = {}   # (hb, pk) -> flat [128, (s, ic, h)] = [128, 4096]
            xt_sb = {}  # (hb, pk) -> flat [128, (s, hc, i)] = [128, 4096]
            for hb in range(NHB):
                for pk in range(NPK):
                    samp0 = hb * 8 + pk * 4
                    xs = xpool.tile([P, PK * 4 * H], F32, tag="x")
                    nc.sync.dma_start(
                        out=ap(xs, [[4 * H, PK], [H, 4], [1, H]]),
                        in_=dram_ap(x_d, [[H, P], [I * H, PK], [P * H, 4], [1, H]],
                                    off=samp0 * I * H),
                    )
                    mark("x", xs)
                    x_sb[(hb, pk)] = xs
                    xt = xtpool.tile([P, PK * 2 * I], F32, tag="xt")
                    for s in range(PK):
                        for hc in range(2):
                            pe_sync("cst", "x", "act", "dve")
                            pxt = ps_xt.tile([P, I], F32, tag="pxt")
                            for ic in range(4):
                                T(
                                    pxt[:, ic * P:(ic + 1) * P],
                                    xs[:, s * 1024 + ic * H + hc * P:
                                           s * 1024 + ic * H + (hc + 1) * P],
                                    id_sb,
                                )
                            dst = xt[:, s * 1024 + hc * I: s * 1024 + (hc + 1) * I]
                            if pk == 0:
                                nc.scalar.activation(dst, pxt[:], AF.Copy)
                                mark("act", dst)
                            else:
                                nc.vector.tensor_copy(dst, pxt[:])
                                mark("dve", dst)
                    xt_sb[(hb, pk)] = xt

            cT = {0: None, 1: None}  # per-hb [128 i%128, (pk, ic, s, o)]
            for hb in range(NHB):
                for t in range(NITER):
                    # ---- y = c @ x : [(pk,) 4s*32o', 256h] ----
                    pe_sync("cst", "x", "dve", "act")
                    ps_y = ps_mid.tile([P, NPK * H], F32, tag="mid")
                    for pk in range(NPK):
                        for s in range(PK):
                            for ic in range(4):
                                lhsT = (c0_sb if t == 0 else
                                        ap(cT[hb], [[1, O]],
                                           off=pk * 512 + ic * P + s * O))
                                MM(
                                    ps_y[32 * s:32 * s + 32,
                                         pk * H:(pk + 1) * H],
                                    lhsT,
                                    ap(x_sb[hb, pk], [[1, H]],
                                       off=s * 1024 + ic * H),
                                    start=(ic == 0),
                                    stop=(ic == 3),
                                    tile_position=(0, 32 * s),
                                )
                    y_sb = work.tile([P, NPK * H], F32, tag="ysb")
                    nc.scalar.activation(y_sb[:], ps_y[:], AF.Copy)
                    mark("act", y_sb)

                    # ---- yT : [h, (pk, 4s*32o')] ----
                    pe_sync("act", "dve")
                    ps_yt = ps_mid.tile([P, NPK * 2 * P], F32, tag="mid")
                    for pk in range(NPK):
                        for hc in range(2):
                            T(
                                ps_yt[:, (pk * 2 + hc) * P:(pk * 2 + hc + 1) * P],
                                y_sb[:, pk * H + hc * P: pk * H + (hc + 1) * P],
                                id_sb,
                            )
                    yt_sb = work.tile([P, NPK * 2 * P], F32, tag="ytsb")
                    nc.vector.tensor_copy(yt_sb[:], ps_yt[:])
                    mark("dve", yt_sb)

                    # ---- vfull = y @ W : [(pk,) 4s*32o', (o,f)] ----
                    pe_sync("dve", "act")
                    ps_vf = ps_big.tile([P, NPK * OF], F32, tag="big")
                    for pk in range(NPK):
                        for hc in range(2):
                            MM(
                                ps_vf[:, pk * OF:(pk + 1) * OF],
                                yt_sb[:, (pk * 2 + hc) * P:(pk * 2 + hc + 1) * P],
                                ap(w_sb, [[1, OF]], off=hc * OF),
                                start=(hc == 0),
                                stop=(hc == 1),
                            )

                    # ---- diag extract: mask, then per-sample partition sum ----
                    msk_sb = work.tile([P, NPK * OF], F32, tag="bigsb")
                    dep(nc.vector.tensor_mul(
                        ap(msk_sb, [[OF, NPK], [1, OF]]),
                        ap(ps_vf, [[OF, NPK], [1, OF]]),
                        ap(mc_sb, [[0, NPK], [1, OF]]),
                    ), dcst_a)
                    mark("dve", msk_sb)
                    pe_sync("dve")
                    ps_vr = ps_big.tile([PK, NPK * OF], F32, tag="big")
                    for pk in range(NPK):
                        MM(
                            ps_vr[:, pk * OF:(pk + 1) * OF],
                            s4_sb,
                            msk_sb[:, pk * OF:(pk + 1) * OF],
                            start=True, stop=True,
                        )

                    # ---- squash: factor = sqrt(mag)/(1+mag), Newton step ----
                    vr_sb = work.tile([PK, NPK * OF], F32, tag="vrsb", bufs=1)
                    nc.scalar.activation(vr_sb[:], ps_vr[:], AF.Copy)
                    mark("act", vr_sb)
                    sq_sb = work.tile([PK, NPK * OF], F32, tag="sqsb", bufs=1)
                    nc.scalar.activation(sq_sb[:], vr_sb[:], AF.Square)
                    mag = sm.tile([PK, NPK * O], F32, tag="mag")
                    nc.vector.reduce_sum(
                        out=mag[:],
                        in_=ap(sq_sb, [[F, NPK * O], [1, F]]),
                        axis=AX.X,
                    )
                    s0 = sm.tile([PK, NPK * O], F32, tag="s0")
                    nc.scalar.activation(s0[:], mag[:], AF.Sqrt)
                    mark("act", s0)
                    r0 = sm.tile([PK, NPK * O], F32, tag="r0")
                    nc.vector.reciprocal(r0[:], s0[:])
                    t1 = sm.tile([PK, NPK * O], F32, tag="t1")
                    nc.vector.tensor_mul(t1[:], mag[:], r0[:])
                    sadd = sm.tile([PK, NPK * O], F32, tag="sadd")
                    nc.vector.tensor_add(sadd[:], s0[:], t1[:])   # ~2 sqrt(mag)
                    onep = sm.tile([PK, NPK * O], F32, tag="onep")
                    nc.vector.tensor_scalar_add(onep[:], mag[:], 1.0)
                    rec = sm.tile([PK, NPK * O], F32, tag="rec")
                    nc.vector.reciprocal(rec[:], onep[:])
                    rec2 = sm.tile([PK, NPK * O], F32, tag="rec2")
                    nc.vector.tensor_scalar_mul(rec2[:], rec[:], 0.5)
                    factor = sm.tile([PK, NPK * O], F32, tag="fac")
                    nc.vector.tensor_mul(factor[:], sadd[:], rec2[:])

                    vsq = work.tile([PK, NPK * OF], F32, tag="vsq", bufs=3)
                    nc.vector.tensor_mul(
                        ap(vsq, [[F, NPK * O], [1, F]]),
                        ap(vr_sb, [[F, NPK * O], [1, F]]),
                        ap(factor, [[1, NPK * O], [0, F]]),
                    )
                    mark("dve", vsq)

                    if t == NITER - 1:
                        nc.sync.dma_start(
                            out=dram_ap(out_d, [[OF, PK], [PK * OF, NPK], [1, OF]],
                                        off=hb * 8 * OF),
                            in_=ap(vsq, [[OF, NPK], [1, OF]]),
                        )
                        continue

                    # ---- vT chunks: [(o8,f16)%128, (pk, m, s)] ----
                    pe_sync("dve", "act")
                    ps_vt = ps_mid.tile([P, NPK * 4 * PK], F32, tag="mid")
                    for pk in range(NPK):
                        for m in range(4):
                            T(
                                ps_vt[:, (pk * 4 + m) * PK:(pk * 4 + m + 1) * PK],
                                vsq[:, pk * OF + m * P: pk * OF + (m + 1) * P],
                                id_sb[:PK, :PK],
                            )
                    vt_sb = work.tile([P, NPK * 4 * PK], F32, tag="vtsb")
                    nc.vector.tensor_copy(vt_sb[:], ps_vt[:])

                    # ---- Vmat blocks: vp[p,(pk,m,s,j)] = vt * blockmask ----
                    vp_sb = work.tile([P, NPK * 4 * PK * O], F32, tag="vp", bufs=1)
                    for m in range(4):
                        nc.vector.tensor_mul(
                            ap(vp_sb, [[4 * PK * O, NPK], [O, PK], [1, O]],
                               off=m * PK * O),
                            ap(vt_sb, [[4 * PK, NPK], [1, PK], [0, O]],
                               off=m * PK),
                            ap(bm_sb, [[0, NPK], [0, PK], [1, O]],
                               off=m * O),
                        )
                    mark("dve", vp_sb[:, 3 * PK * O: 3 * PK * O + 1])

                    # ---- z = Vmat @ WT : [(pk,) 4s*32o, 256h] ----
                    pe_sync("dve", "act")
                    ps_z = ps_mid.tile([P, NPK * H], F32, tag="mid")
                    for pk in range(NPK):
                        for s in range(PK):
                            for m in range(4):
                                MM(
                                    ps_z[32 * s:32 * s + 32,
                                         pk * H:(pk + 1) * H],
                                    ap(vp_sb, [[1, O]],
                                       off=pk * 512 + m * P + s * O),
                                    ap(wt_sb, [[1, H]], off=m * H),
                                    start=(m == 0),
                                    stop=(m == 3),
                                    tile_position=(0, 32 * s),
                                )
                    z_sb = work.tile([P, NPK * H], F32, tag="ysb")
                    nc.scalar.activation(z_sb[:], ps_z[:], AF.Copy)
                    mark("act", z_sb)

                    # ---- zT ----
                    pe_sync("act", "dve")
                    ps_zt = ps_mid.tile([P, NPK * 2 * P], F32, tag="mid")
                    for pk in range(NPK):
                        for hc in range(2):
                            T(
                                ps_zt[:, (pk * 2 + hc) * P:(pk * 2 + hc + 1) * P],
                                z_sb[:, pk * H + hc * P: pk * H + (hc + 1) * P],
                                id_sb,
                            )
                    zt_sb = work.tile([P, NPK * 2 * P], F32, tag="ytsb")
                    nc.vector.tensor_copy(zt_sb[:], ps_zt[:])
                    mark("dve", zt_sb)

                    # ---- b = z @ xT : [(pk,) 4s*32o, 512i] ----
                    pe_sync("dve", "act")
                    ps_b = ps_big.tile([P, NPK * I], F32, tag="big")
                    for pk in range(NPK):
                        for s in range(PK):
                            for hc in range(2):
                                MM(
                                    ps_b[32 * s:32 * s + 32,
                                         pk * I:(pk + 1) * I],
                                    ap(zt_sb, [[1, O]],
                                       off=(pk * 2 + hc) * P + 32 * s),
                                    ap(xt_sb[hb, pk], [[1, I]],
                                       off=s * 1024 + hc * I),
                                    start=(hc == 0),
                                    stop=(hc == 1),
                                    tile_position=(0, 32 * s),
                                )

                    # ---- softmax over o (b in +-5 => exp w/o max-subtract) ----
                    expb = work.tile([P, NPK * I], F32, tag="bigsb")
                    nc.scalar.activation(expb[:], ps_b[:], AF.Exp)
                    mark("act", expb)

                    pe_sync("act", "dve")
                    ps_ebt = ps_big.tile([P, NPK * 4 * P], F32, tag="big")
                    for pk in range(NPK):
                        for ic in range(4):
                            T(
                                ps_ebt[:, (pk * 4 + ic) * P:(pk * 4 + ic + 1) * P],
                                expb[:, pk * I + ic * P: pk * I + (ic + 1) * P],
                                id_sb,
                            )
                    ebt = work.tile([P, NPK * 4 * P], F32, tag="ebt", bufs=1)
                    nc.scalar.activation(ebt[:], ps_ebt[:], AF.Copy)
                    mark("act", ebt)

                    ssum = sm.tile([P, NPK * 4 * PK], F32, tag="ssum")
                    nc.vector.reduce_sum(
                        out=ssum[:],
                        in_=ap(ebt, [[O, NPK * 4 * PK], [1, O]]),
                        axis=AX.X,
                    )
                    rsum = sm.tile([P, NPK * 4 * PK], F32, tag="rsum")
                    nc.vector.reciprocal(rsum[:], ssum[:])
                    cT[hb] = work.tile([P, NPK * 4 * P], F32, tag="ct%d" % hb,
                                       bufs=1, name="ct_t")
                    nc.vector.tensor_mul(
                        ap(cT[hb], [[O, NPK * 4 * PK], [1, O]]),
                        ap(ebt, [[O, NPK * 4 * PK], [1, O]]),
                        ap(rsum, [[1, NPK * 4 * PK], [0, O]]),
                    )
                    mark("dve", cT[hb])

    if split_waits:
        _split_fat_waits(nc)
    return nc


def _split_fat_waits(nc, maxw=1):
    """Walrus caps sync waits per instruction; split overflow onto extra
    same-engine Drain instructions inserted just before the offender."""
    nsplit = 0
    for blk in nc.m.functions[0].blocks:
        new_insts = []
        for inst in blk.instructions:
            si = getattr(inst, "sync_info", None)
            w = list(si.on_wait) if si is not None and si.on_wait else []
            if len(w) > maxw:
                for k in range(0, len(w) - maxw, maxw):
                    d = mybir.InstDrain(name="I-waitsplit-%d" % nsplit,
                                        ins=[], outs=[])
                    nsplit += 1
                    d.engine = inst.engine
                    d.sync_info = mybir.SyncInfo(on_wait=w[k:k + maxw],
                                                 on_update=[])
                    new_insts.append(d)
                si.on_wait = w[len(w) - maxw:]
            new_insts.append(inst)
        blk.instructions[:] = new_insts
    return nc


_NC_CACHE = None


def make_cst(Wn):
    """Constant blob [128, CSTN] matching the device-side layout."""
    cst = np.zeros((P, CSTN), np.float32)
    # W [h, of] -> [h%128, (hc, of)]
    cst[:, CW:CW + 2 * OF] = Wn.reshape(2, P, OF).transpose(1, 0, 2).reshape(P, 2 * OF)
    # WT [of, h] -> [of%128, (m, h)]
    cst[:, CWT:CWT + 4 * H] = (
        Wn.T.reshape(4, P, H).transpose(1, 0, 2).reshape(P, 4 * H))
    cst[:, CID:CID + P] = np.eye(P, dtype=np.float32)
    for p in range(P):
        o = p % O
        cst[p, CMC + o * F:CMC + (o + 1) * F] = 1.0
    cst[np.arange(P), CS4 + np.arange(P) // O] = 1.0
    for p in range(P):
        for m in range(4):
            cst[p, CBM + m * O + 8 * m + p // F] = 1.0
    cst[:, CC0:CC0 + O] = 1.0 / O
    return cst


def make_in_maps(x, W):
    x = np.ascontiguousarray(np.asarray(x, dtype=np.float32))
    Wn = np.ascontiguousarray(np.asarray(W, dtype=np.float32).reshape(H, OF))
    cst = make_cst(Wn)
    xs = x.reshape(NCORES, S, I, H)
    return [
        {"x": np.ascontiguousarray(xs[c]), "cst": cst}
        for c in range(NCORES)
    ]


def kernel(x: np.ndarray, W: np.ndarray) -> np.ndarray:
    global _NC_CACHE
    if _NC_CACHE is None:
        _NC_CACHE = build_program()
    in_maps = make_in_maps(x, W)
    res = run_bass_kernel_spmd(_NC_CACHE, in_maps, core_ids=list(range(NCORES)))
    out = np.stack([res.results[c]["out"] for c in range(NCORES)])
    return out.reshape(B, O, F)



# revision 2
# speedup vs baseline: 1.0449x; 1.0449x over previous
"""Trainium2 Bass kernel for nn_Capsule (dynamic routing), bf16 dataflow.

reference: u = x @ W  (per-sample [512,256]@[256,512]); b=0
           3x { c = softmax_o(b); v[o,f] = sum_i c[o,i] u[i,(o,f)];
                v = squash(v); b[o,i] = sum_f v[o,f] u[i,(o,f)] }
           return v [B, 32, 16]

u is never materialized.  Per core: 16 samples = 4 quads of 4.
All matmuls in bf16 (1 cyc/row, fast LDWEIGHTS); fp32 only in PSUM and
the squash scalar chain.  Host ships x twice (natural + pre-transposed)
in bf16, so no on-device transposes of x are needed.

Per (iter t, quad q), layouts ([partition, free]):
  cT   [i%128, (ic4, s4, o32)]  bf16   (t=0: uniform 1/32)
  y    = cT.T @ x          -> ps_y  [(s,o)128, h256]      (16 MM ap256)
  yT   via 2 PE transposes -> yt_sb [h%128, (hc2, so128)] bf16
  vfT  = W @ yT            -> ps_vf [of%128, (g4, so128)] (8 MM ap128)
  diag: mask-mul (gpsimd) + free-reduce over o' (DVE) -> vr [of%128,(g4,s4)] f32
  mag  = I16.T @ vr^2 (PE) -> [o-sub 8, (g,s)16]; factor = exp(.5 ln m)/(1+m)
         (Ln+Exp share one act-table set -> no table reloads)
  fac128 = E8.T @ factor (PE broadcast over f partitions)
  vsq  = vr * fac128 -> bf16
  VmatT: vsq bcast over o' * mask (gpsimd) -> vp [of%128, (g4,s4,o'32)] bf16
  zT   = WT @ VmatT        -> ps_zt [h%128, (hc2, so128)] (8 MM ap128)
  b    = zt.T @ xT         -> ps_b  [(s,o)128, i512]      (8 MM ap512)
  eb   = exp(b) (Act, bf16); ebT via 4 PE transposes (bf16 PSUM)
  softmax over o in [i, (ic,s,o)] layout -> cT for next iter
Last iter stops after vsq; output transposed on PE and cast bf16->f32
by a gpsimd DMA.
"""

import numpy as np
import ml_dtypes

import concourse.bass as bass
import concourse.tile as tile
from concourse import mybir
from concourse.bass_utils import run_bass_kernel_spmd

F32 = mybir.dt.float32
BF16 = mybir.dt.bfloat16
AF = mybir.ActivationFunctionType
AX = mybir.AxisListType

B, I, H = 128, 512, 256
O, F = 32, 16
OF = O * F            # 512
NCORES = 8
S = B // NCORES       # 16 samples per core
Q = 4                 # quads per core
QS = 4                # samples per quad
NIT = 3
P = 128

# bf16 constant blob [128, CSTN]
CW = 0                # W  [h%128, (hc2, of512)]
CWT = CW + 2 * OF     # WT [of%128, (m4, h256)]
CID = CWT + 4 * H     # identity [128, 128]
CMD = CID + P         # diag mask [128, (g4, o32)]: md[p,(g,o)] = (o == g*8+p//16)
CI16 = CMD + 4 * O    # [128, 8]: i16[p, j] = (j == p//16)
CE8 = CI16 + 8        # [8, 128]: e8[j, p] = (p//16 == j)
CC0 = CE8 + P         # [128, 32] = 1/32
CIDF = CC0 + O        # 2 bf16 cols whose bytes alias to f32 1.0
CSTN = CIDF + 2


def ap(t, dims, off=0):
    """AP over tile/handle `t`: keep partition dim, explicit free dims."""
    a = t if isinstance(t, bass.AP) else t[:]
    return bass.AP(tensor=a.tensor, offset=a.offset + off,
                   ap=[list(a.ap[0])] + [list(d) for d in dims])


def f32view(a, off=0):
    """1x1 f32 alias of an SBUF AP's base (for sync-anchor reads only)."""
    t = a.tensor
    t2 = t if t.dtype == F32 else bass.SBTensorHandle(
        name=t.name, shape=[t.shape[0], t.shape[1] // 2], dtype=F32,
        base_partition=t.base_partition,
        manual_sbuf_range=t.manual_sbuf_range,
        manual_base_name=t.manual_base_name)
    return bass.AP(tensor=t2, offset=off,
                   ap=[[int(t2.shape[1]), 1], [1, 1]])


def dram_ap(handle, dims, off=0):
    """AP over DRAM handle with fully explicit dims (first = partition)."""
    a = handle[:]
    return bass.AP(tensor=a.tensor, offset=a.offset + off,
                   ap=[list(d) for d in dims])


MM_LABELS = []


def build_program(split_waits=True):
    MM_LABELS.clear()
    nc = bass.Bass("TRN2", target_bir_lowering=False)

    x_d = nc.dram_tensor("xb", [S, I, H], BF16, kind="ExternalInput")
    xt_d = nc.dram_tensor("xtb", [S, H, I], BF16, kind="ExternalInput")
    cst_d = nc.dram_tensor("cst", [P, CSTN], BF16, kind="ExternalInput")
    out_d = nc.dram_tensor("out", [S, OF], F32, kind="ExternalOutput")

    with tile.TileContext(nc) as tc:
        with (
            tc.tile_pool(name="consts", bufs=1) as consts,
            tc.tile_pool(name="xpool", bufs=4) as xpool,
            tc.tile_pool(name="xtpool", bufs=4) as xtpool,
            tc.tile_pool(name="work", bufs=2) as work,
            tc.tile_pool(name="ps", bufs=1, space="PSUM") as ps,
        ):
            cst = consts.tile([P, CSTN], BF16)
            nc.sync.dma_start(out=cst[:], in_=cst_d[:])
            w_sb = cst[:, CW:CW + 2 * OF]
            wt_sb = cst[:, CWT:CWT + 4 * H]
            id_sb = cst[:, CID:CID + P]
            md_sb = cst[:, CMD:CMD + 4 * O]
            i16_sb = cst[:, CI16:CI16 + 8]
            e8_sb = cst[:8, CE8:CE8 + P]
            c0_sb = cst[:, CC0:CC0 + O]

            # PE sync anchors: PE observes foreign engine clocks via 1x1
            # transposes so walrus can elide per-instruction waits.
            anch = ps.tile([P, 16], F32, tag="anch")
            idf1 = f32view(cst[:], off=CIDF // 2)
            dirty = {}
            acol = [0]
            pending = []

            def mark(key, apv):
                dirty[key] = apv

            def pe_sync(*keys, force=False):
                # Anchors proved to over-serialize (PE waits on the globally
                # newest op of an engine, not the actual dependency); rely on
                # Tile's precise per-instruction deps instead.
                pending.clear()
                if not force:
                    return
                for k in keys:
                    if k not in dirty:
                        continue
                    d = dirty.pop(k)
                    MM_LABELS.append("anch:" + k)
                    a = nc.tensor.transpose(
                        anch[:1, acol[0]:acol[0] + 1], f32view(d),
                        idf1)
                    pending.append(a.ins)
                    acol[0] = (acol[0] + 1) % 16

            def _chain(b):
                for a in pending:
                    bass._add_dep_helper(b.ins, a, sync=False,
                                         reason="pe-anchor order")
                return b

            def T(out, in_, ident, label=""):
                MM_LABELS.append(label or CUR[0] + ":T")
                return _chain(nc.tensor.transpose(out, in_, ident))

            def MM(out, lhsT, rhs, label="", **kw):
                MM_LABELS.append(label or CUR[0])
                return _chain(nc.tensor.matmul(out, lhsT, rhs, **kw))

            mark("cst", cst)
            CUR = ["init"]

            # ---- input DMAs (x natural + pre-transposed, interleaved so
            # quad q's xT lands before its first b-stage) ----
            x_sb = {}
            xt_sb = {}

            def load_x(q):
                xs = xpool.tile([P, QS * 4 * H], BF16, tag="x")
                nc.sync.dma_start(
                    out=ap(xs, [[4 * H, QS], [H, 4], [1, H]]),
                    in_=dram_ap(x_d, [[H, P], [I * H, QS], [P * H, 4], [1, H]],
                                off=q * QS * I * H),
                )
                mark("x%d" % q, xs)
                x_sb[q] = xs

            def load_xt(q):
                xt = xtpool.tile([P, QS * 2 * I], BF16, tag="xt")
                nc.sync.dma_start(
                    out=ap(xt, [[2 * I, QS], [I, 2], [1, I]]),
                    in_=dram_ap(xt_d, [[I, P], [H * I, QS], [P * I, 2], [1, I]],
                                off=q * QS * H * I),
                )
                mark("xt%d" % q, xt)
                xt_sb[q] = xt

            load_x(0)
            load_x(1)
            load_xt(0)
            load_x(2)
            load_xt(1)
            load_x(3)
            load_xt(2)
            load_xt(3)

            cT = {q: None for q in range(Q)}

            def mk_chunks(q):
                """Per-quad list of emission chunks; wavefront-interleaved
                across quads so PE always has another quad's work during
                cross-engine chain latencies."""
                st = {}
                chunks = []
                for t in range(NIT):
                    chunks.extend(stage_fns(q, t, st))
                return chunks

            def stage_fns(q, t, st):
                last = (t == NIT - 1)

                def A():
                    CUR[0] = "A.q%d.t%d" % (q, t)
                    pe_sync("cst", "x%d" % q, "dve")
                    ps_y = ps.tile([P, H], F32, tag="y")
                    for s in range(QS):
                        for ic in range(4):
                            lhsT = (c0_sb if t == 0 else
                                    ap(cT[q], [[1, O]], off=ic * P + s * O))
                            MM(
                                ps_y[32 * s:32 * s + 32, :],
                                lhsT,
                                ap(x_sb[q], [[1, H]], off=s * 4 * H + ic * H),
                                start=(ic == 0),
                                stop=(ic == 3),
                                tile_position=(0, 32 * s),
                            )
                    ysb = work.tile([P, H], BF16, tag="ysb", bufs=4)
                    nc.scalar.activation(ysb[:], ps_y[:], AF.Copy)
                    mark("act", ysb)
                    st["ysb"] = ysb

                def Bc():
                    CUR[0] = "Bc.q%d.t%d" % (q, t)
                    pe_sync("act")
                    ps_yt = ps.tile([P, 4 * P], BF16, tag="ytb")
                    for hc in range(2):
                        T(ps_yt[:, hc * P:(hc + 1) * P],
                          st["ysb"][:, hc * P:(hc + 1) * P], id_sb)
                    ytsb = work.tile([P, 2 * P], BF16, tag="ytsb", bufs=4)
                    nc.vector.tensor_copy(ytsb[:], ps_yt[:, :2 * P])
                    mark("dve", ytsb)
                    st["ytsb"] = ytsb

                def C():
                    CUR[0] = "C.q%d.t%d" % (q, t)
                    pe_sync("dve")
                    ps_vf = ps.tile([P, 4 * P], F32, tag="vf", bufs=2)
                    for g in range(4):
                        for hc in range(2):
                            MM(
                                ps_vf[:, g * P:(g + 1) * P],
                                ap(w_sb, [[1, P]], off=hc * OF + g * P),
                                ap(st["ytsb"], [[1, P]], off=hc * P),
                                start=(hc == 0),
                                stop=(hc == 1),
                            )
                    msk = work.tile([P, 4 * P], BF16, tag="msk")
                    nc.vector.tensor_mul(
                        ap(msk, [[P, 4], [O, QS], [1, O]]),
                        ap(ps_vf, [[P, 4], [O, QS], [1, O]]),
                        ap(md_sb, [[O, 4], [0, QS], [1, O]]),
                    )
                    vr = work.tile([P, 16], F32, tag="vr", bufs=4)
                    nc.vector.reduce_sum(
                        out=vr[:],
                        in_=ap(msk, [[O, 16], [1, O]]),
                        axis=AX.X,
                    )
                    vrsq = work.tile([P, 16], BF16, tag="vrsq", bufs=4)
                    nc.vector.tensor_mul(vrsq[:], vr[:], vr[:])
                    mark("dve", vrsq)
                    st["vr"] = vr
                    st["vrsq"] = vrsq

                def DE():
                    CUR[0] = "DE.q%d.t%d" % (q, t)
                    pe_sync("dve")
                    mf = ps.tile([P, 32], F32, tag="mf")
                    MM(mf[:8, :16], i16_sb, st["vrsq"][:])
                    lnm = work.tile([8, 16], F32, tag="lnm")
                    nc.scalar.activation(lnm[:], mf[:8, :16], AF.Ln)
                    s0 = work.tile([8, 16], F32, tag="s0")
                    nc.scalar.activation(s0[:], lnm[:], AF.Exp, scale=0.5)
                    mark("act", s0)
                    onep = work.tile([8, 16], F32, tag="onep")
                    nc.vector.tensor_scalar_add(onep[:], mf[:8, :16], 1.0)
                    rp = work.tile([8, 16], F32, tag="rp")
                    nc.vector.reciprocal(rp[:], onep[:])
                    facb = work.tile([8, 16], BF16, tag="facb", bufs=4)
                    nc.vector.tensor_mul(facb[:], s0[:], rp[:])
                    mark("dve", facb)
                    pe_sync("dve", "act")
                    MM(mf[:, 16:32], e8_sb, facb[:8, :])
                    if not last:
                        vsq = work.tile([P, 16], BF16, tag="vsq", bufs=4)
                        nc.vector.tensor_mul(vsq[:], st["vr"][:],
                                             mf[:, 16:32])
                        vp = work.tile([P, 4 * P], BF16, tag="vp", bufs=4)
                        nc.vector.tensor_mul(
                            ap(vp, [[P, 4], [O, QS], [1, O]]),
                            ap(vsq, [[4, 4], [1, QS], [0, O]]),
                            ap(md_sb, [[O, 4], [0, QS], [1, O]]),
                        )
                        mark("dve", vp)
                        st["vp"] = vp
                    else:
                        vsq = work.tile([P, 16], BF16, tag="vsq", bufs=4)
                        nc.vector.tensor_mul(
                            ap(vsq, [[1, 4], [4, 4]]),
                            ap(st["vr"], [[4, 4], [1, 4]]),
                            ap(mf, [[4, 4], [1, 4]], off=16),
                        )
                        mark("dve", vsq)
                        pe_sync("dve")
                        ps_vo = ps.tile([P, 4 * P], BF16, tag="ytb")
                        T(ps_vo[:16, :P], vsq[:], id_sb)
                        vo = work.tile([16, P], BF16, tag="vo")
                        nc.vector.tensor_copy(vo[:], ps_vo[:16, :P])
                        nc.gpsimd.dma_start(
                            out=dram_ap(out_d, [[P, 16], [1, P]],
                                        off=q * QS * OF),
                            in_=vo[:],
                        )

                def Fc():
                    CUR[0] = "Fc.q%d.t%d" % (q, t)
                    pe_sync("dve")
                    ps_zt = ps.tile([P, 2 * P], F32, tag="zt")
                    for hc in range(2):
                        for m in range(4):
                            MM(
                                ps_zt[:, hc * P:(hc + 1) * P],
                                ap(wt_sb, [[1, P]], off=m * H + hc * P),
                                ap(st["vp"], [[1, P]], off=m * P),
                                start=(m == 0),
                                stop=(m == 3),
                            )
                    ztsb = work.tile([P, 2 * P], BF16, tag="ztsb", bufs=4)
                    nc.scalar.activation(ztsb[:], ps_zt[:], AF.Copy)
                    mark("act", ztsb)
                    st["ztsb"] = ztsb

                def G():
                    CUR[0] = "G.q%d.t%d" % (q, t)
                    pe_sync("act", "xt%d" % q)
                    ps_b = ps.tile([P, I], F32, tag="b")
                    for s in range(QS):
                        for hc in range(2):
                            MM(
                                ps_b[32 * s:32 * s + 32, :],
                                ap(st["ztsb"], [[1, O]], off=hc * P + s * O),
                                ap(xt_sb[q], [[1, I]], off=s * 2 * I + hc * I),
                                start=(hc == 0),
                                stop=(hc == 1),
                                tile_position=(0, 32 * s),
                            )
                    eb = work.tile([P, I], BF16, tag="eb", bufs=4)
                    nc.scalar.activation(eb[:], ps_b[:], AF.Exp)
                    mark("act", eb)
                    st["eb"] = eb

                def Hc():
                    CUR[0] = "Hc.q%d.t%d" % (q, t)
                    pe_sync("act")
                    ps_ebt = ps.tile([P, 4 * P], BF16, tag="ytb")
                    for ic in range(4):
                        T(ps_ebt[:, ic * P:(ic + 1) * P],
                          st["eb"][:, ic * P:(ic + 1) * P], id_sb)
                    ssum = work.tile([P, 16], F32, tag="ssum")
                    nc.vector.reduce_sum(
                        out=ssum[:],
                        in_=ap(ps_ebt, [[O, 16], [1, O]]),
                        axis=AX.X,
                    )
                    rs = work.tile([P, 16], F32, tag="rs")
                    nc.vector.reciprocal(rs[:], ssum[:])
                    rsb = work.tile([P, 16], BF16, tag="rsb")
                    nc.vector.tensor_copy(rsb[:], rs[:])
                    ct = work.tile([P, 4 * P], BF16, tag="ct%d" % q, bufs=1)
                    nc.vector.tensor_mul(
                        ap(ct, [[O, 16], [1, O]]),
                        ap(ps_ebt, [[O, 16], [1, O]]),
                        ap(rsb, [[1, 16], [0, O]]),
                    )
                    mark("dve", ct)
                    cT[q] = ct

                if last:
                    return [A, Bc, C, DE]
                return [A, Bc, C, DE, Fc, G, Hc]

            all_chunks = {q: mk_chunks(q) for q in range(Q)}
            L = len(all_chunks[0])
            for k in range(L + Q - 1):
                for q in reversed(range(Q)):
                    c = k - q
                    if 0 <= c < L:
                        all_chunks[q][c]()

            if False:
                pass

                # ======== stage A: y, then yT ========
                y_sbq = {}
                yt_sbq = {}
                for q in range(Q):
                    pe_sync("cst", "x%d" % q, "dve")
                    ps_y = ps.tile([P, H], F32, tag="y")
                    for s in range(QS):
                        for ic in range(4):
                            lhsT = (c0_sb if t == 0 else
                                    ap(cT[q], [[1, O]], off=ic * P + s * O))
                            MM(
                                ps_y[32 * s:32 * s + 32, :],
                                lhsT,
                                ap(x_sb[q], [[1, H]], off=s * 4 * H + ic * H),
                                start=(ic == 0),
                                stop=(ic == 3),
                                tile_position=(0, 32 * s),
                            )
                    ysb = work.tile([P, H], BF16, tag="ysb", bufs=4)
                    nc.scalar.activation(ysb[:], ps_y[:], AF.Copy)
                    mark("act", ysb)
                    y_sbq[q] = ysb
                for q in range(Q):
                    pe_sync("act")
                    ps_yt = ps.tile([P, 4 * P], BF16, tag="ytb")
                    for hc in range(2):
                        T(ps_yt[:, hc * P:(hc + 1) * P],
                          y_sbq[q][:, hc * P:(hc + 1) * P], id_sb)
                    ytsb = work.tile([P, 2 * P], BF16, tag="ytsb", bufs=4)
                    nc.vector.tensor_copy(ytsb[:], ps_yt[:, :2 * P])
                    mark("dve", ytsb)
                    yt_sbq[q] = ytsb

                # ======== stage B (sw-pipelined by one quad) ========
                # per q: vfT (PE) -> msk (gpsimd) -> vr,vrsq (DVE)
                #        -> mag (PE) -> ln/exp (Act) -> 1+m,recip,fac (DVE)
                #        -> [q-1: fac128 (PE) -> vsq (DVE) -> vp (gpsimd)]
                vr_q = {}
                mf_q = {}
                fac_q = {}
                vp_q = {}

                def emit_tail(qq):
                    # fac128 broadcast, vsq; VmatT only when not last iter
                    pe_sync("dve", "act")
                    MM(mf_q[qq][:, 16:32], e8_sb, fac_q[qq][:8, :])
                    if not last:
                        vsq = work.tile([P, 16], BF16, tag="vsq", bufs=4)
                        nc.vector.tensor_mul(vsq[:], vr_q[qq][:],
                                             mf_q[qq][:, 16:32])
                        mark("dve", vsq)
                        vp = work.tile([P, 4 * P], BF16, tag="vp", bufs=4)
                        nc.gpsimd.tensor_mul(
                            ap(vp, [[P, 4], [O, QS], [1, O]]),
                            ap(vsq, [[4, 4], [1, QS], [0, O]]),
                            ap(md_sb, [[O, 4], [0, QS], [1, O]]),
                        )
                        mark("pool", vp)
                        vp_q[qq] = vp
                    else:
                        # (s,g) free order so the PE transpose lands the
                        # output with a single uniform DRAM stride
                        vsq = work.tile([P, 16], BF16, tag="vsq", bufs=4)
                        nc.vector.tensor_mul(
                            ap(vsq, [[1, 4], [4, 4]]),
                            ap(vr_q[qq], [[4, 4], [1, 4]]),
                            ap(mf_q[qq], [[4, 4], [1, 4]], off=16),
                        )
                        mark("dve", vsq)
                        pe_sync("dve")
                        ps_vo = ps.tile([P, 4 * P], BF16, tag="ytb")
                        T(ps_vo[:16, :P], vsq[:], id_sb)
                        vo = work.tile([16, P], BF16, tag="vo")
                        nc.vector.tensor_copy(vo[:], ps_vo[:16, :P])
                        nc.gpsimd.dma_start(
                            out=dram_ap(out_d, [[P, 16], [1, P]],
                                        off=qq * QS * OF),
                            in_=vo[:],
                        )

                for q in range(Q):
                    pe_sync("dve")
                    ps_vf = ps.tile([P, 4 * P], F32, tag="vf", bufs=2)
                    for g in range(4):
                        for hc in range(2):
                            MM(
                                ps_vf[:, g * P:(g + 1) * P],
                                ap(w_sb, [[1, P]], off=hc * OF + g * P),
                                ap(yt_sbq[q], [[1, P]], off=hc * P),
                                start=(hc == 0),
                                stop=(hc == 1),
                            )
                    msk = work.tile([P, 4 * P], BF16, tag="msk")
                    nc.vector.tensor_mul(
                        ap(msk, [[P, 4], [O, QS], [1, O]]),
                        ap(ps_vf, [[P, 4], [O, QS], [1, O]]),
                        ap(md_sb, [[O, 4], [0, QS], [1, O]]),
                    )
                    mark("dve", msk)
                    vr = work.tile([P, 16], F32, tag="vr", bufs=4)
                    nc.vector.reduce_sum(
                        out=vr[:],
                        in_=ap(msk, [[O, 16], [1, O]]),
                        axis=AX.X,
                    )
                    vrsq = work.tile([P, 16], BF16, tag="vrsq", bufs=4)
                    nc.vector.tensor_mul(vrsq[:], vr[:], vr[:])
                    mark("dve", vrsq)
                    vr_q[q] = vr

                    if q > 0:
                        emit_tail(q - 1)

                    pe_sync("dve", "pool")
                    mf = ps.tile([P, 32], F32, tag="mf")
                    MM(mf[:8, :16], i16_sb, vrsq[:])
                    mf_q[q] = mf

                    lnm = work.tile([8, 16], F32, tag="lnm")
                    nc.scalar.activation(lnm[:], mf[:8, :16], AF.Ln)
                    s0 = work.tile([8, 16], F32, tag="s0")
                    nc.scalar.activation(s0[:], lnm[:], AF.Exp, scale=0.5)
                    mark("act", s0)
                    onep = work.tile([8, 16], F32, tag="onep")
                    nc.vector.tensor_scalar_add(onep[:], mf[:8, :16], 1.0)
                    rp = work.tile([8, 16], F32, tag="rp")
                    nc.vector.reciprocal(rp[:], onep[:])
                    facb = work.tile([8, 16], BF16, tag="facb", bufs=4)
                    nc.vector.tensor_mul(facb[:], s0[:], rp[:])
                    mark("dve", facb)
                    fac_q[q] = facb
                emit_tail(Q - 1)

                if last:
                    continue

                # ======== zT + PSUM->SBUF copy ========
                zt_sbq = {}
                for q in range(Q):
                    pe_sync("pool", "dve")
                    ps_zt = ps.tile([P, 2 * P], F32, tag="zt")
                    for hc in range(2):
                        for m in range(4):
                            MM(
                                ps_zt[:, hc * P:(hc + 1) * P],
                                ap(wt_sb, [[1, P]], off=m * H + hc * P),
                                ap(vp_q[q], [[1, P]], off=m * P),
                                start=(m == 0),
                                stop=(m == 3),
                            )
                    ztsb = work.tile([P, 2 * P], BF16, tag="ztsb", bufs=4)
                    nc.scalar.activation(ztsb[:], ps_zt[:], AF.Copy)
                    mark("act", ztsb)
                    zt_sbq[q] = ztsb

                # ======== stage C: b -> exp -> ebT -> softmax ========
                eb_sbq = {}
                for q in range(Q):
                    pe_sync("act", "xt%d" % q)
                    ps_b = ps.tile([P, I], F32, tag="b")
                    for s in range(QS):
                        for hc in range(2):
                            MM(
                                ps_b[32 * s:32 * s + 32, :],
                                ap(zt_sbq[q], [[1, O]], off=hc * P + s * O),
                                ap(xt_sb[q], [[1, I]], off=s * 2 * I + hc * I),
                                start=(hc == 0),
                                stop=(hc == 1),
                                tile_position=(0, 32 * s),
                            )
                    eb = work.tile([P, I], BF16, tag="eb", bufs=4)
                    nc.scalar.activation(eb[:], ps_b[:], AF.Exp)
                    mark("act", eb)
                    eb_sbq[q] = eb
                for q in range(Q):
                    pe_sync("act")
                    ps_ebt = ps.tile([P, 4 * P], BF16, tag="ytb")
                    for ic in range(4):
                        T(ps_ebt[:, ic * P:(ic + 1) * P],
                          eb_sbq[q][:, ic * P:(ic + 1) * P], id_sb)
                    ssum = work.tile([P, 16], F32, tag="ssum")
                    nc.vector.reduce_sum(
                        out=ssum[:],
                        in_=ap(ps_ebt, [[O, 16], [1, O]]),
                        axis=AX.X,
                    )
                    rs = work.tile([P, 16], F32, tag="rs")
                    nc.vector.reciprocal(rs[:], ssum[:])
                    rsb = work.tile([P, 16], BF16, tag="rsb")
                    nc.vector.tensor_copy(rsb[:], rs[:])
                    ct = work.tile([P, 4 * P], BF16, tag="ct%d" % q, bufs=1)
                    nc.vector.tensor_mul(
                        ap(ct, [[O, 16], [1, O]]),
                        ap(ps_ebt, [[O, 16], [1, O]]),
                        ap(rsb, [[1, 16], [0, O]]),
                    )
                    mark("dve", ct)
                    cT[q] = ct

    if split_waits:
        _split_fat_waits(nc)
    return nc


def _split_fat_waits(nc, maxw=1):
    """Walrus caps sync waits per instruction; split overflow onto extra
    same-engine Drain instructions inserted just before the offender."""
    nsplit = 0
    for blk in nc.m.functions[0].blocks:
        new_insts = []
        for inst in blk.instructions:
            si = getattr(inst, "sync_info", None)
            w = list(si.on_wait) if si is not None and si.on_wait else []
            if len(w) > maxw:
                for k in range(0, len(w) - maxw, maxw):
                    d = mybir.InstDrain(name="I-waitsplit-%d" % nsplit,
                                        ins=[], outs=[])
                    nsplit += 1
                    d.engine = inst.engine
                    d.sync_info = mybir.SyncInfo(on_wait=w[k:k + maxw],
                                                 on_update=[])
                    new_insts.append(d)
                si.on_wait = w[len(w) - maxw:]
            new_insts.append(inst)
        blk.instructions[:] = new_insts
    return nc


_NC_CACHE = None


def make_cst(Wn):
    """bf16 constant blob [128, CSTN] matching the device-side layout."""
    cst = np.zeros((P, CSTN), np.float32)
    cst[:, CW:CW + 2 * OF] = (
        Wn.reshape(2, P, OF).transpose(1, 0, 2).reshape(P, 2 * OF))
    cst[:, CWT:CWT + 4 * H] = (
        Wn.T.reshape(4, P, H).transpose(1, 0, 2).reshape(P, 4 * H))
    cst[:, CID:CID + P] = np.eye(P, dtype=np.float32)
    for p in range(P):
        for g in range(4):
            cst[p, CMD + g * O + g * 8 + p // 16] = 1.0
    cst[np.arange(P), CI16 + np.arange(P) // 16] = 1.0
    for j in range(8):
        cst[j, CE8 + 16 * j:CE8 + 16 * (j + 1)] = 1.0
    cst[:, CC0:CC0 + O] = 1.0 / O
    out = cst.astype(ml_dtypes.bfloat16)
    # bf16 pair (0.0, 1.0) little-endian == f32 1.0 when viewed 4-byte
    out[:, CIDF] = ml_dtypes.bfloat16(0.0)
    out[:, CIDF + 1] = ml_dtypes.bfloat16(1.0)
    return out


def make_in_maps(x, W):
    x = np.asarray(x, dtype=np.float32)
    Wn = np.asarray(W, dtype=np.float32).reshape(H, OF)
    cst = make_cst(Wn)
    xb = x.astype(ml_dtypes.bfloat16).reshape(NCORES, S, I, H)
    xtb = np.ascontiguousarray(
        x.reshape(NCORES, S, I, H).transpose(0, 1, 3, 2)
    ).astype(ml_dtypes.bfloat16)
    return [
        {"xb": np.ascontiguousarray(xb[c]),
         "xtb": np.ascontiguousarray(xtb[c]),
         "cst": cst}
        for c in range(NCORES)
    ]


def kernel(x: np.ndarray, W: np.ndarray) -> np.ndarray:
    global _NC_CACHE
    if _NC_CACHE is None:
        _NC_CACHE = build_program()
    in_maps = make_in_maps(x, W)
    res = run_bass_kernel_spmd(_NC_CACHE, in_maps, core_ids=list(range(NCORES)))
    out = np.stack([res.results[c]["out"] for c in range(NCORES)])
    return out.reshape(B, O, F)


# revision 3
# speedup vs baseline: 1.1413x; 1.0923x over previous
"""Trainium2 Bass kernel for nn_Capsule (dynamic routing), bf16 dataflow.

reference: u = x @ W  (per-sample [512,256]@[256,512]); b=0
           3x { c = softmax_o(b); v[o,f] = sum_i c[o,i] u[i,(o,f)];
                v = squash(v); b[o,i] = sum_f v[o,f] u[i,(o,f)] }
           return v [B, 32, 16]

u is never materialized.  Per core: 16 samples = 4 quads of 4.
All matmuls in bf16 (1 cyc/row, fast LDWEIGHTS); fp32 only in PSUM and
the squash scalar chain.  Host ships x twice (natural + pre-transposed)
in bf16, so no on-device transposes of x are needed.

Per (iter t, quad q), layouts ([partition, free]):
  cT   [i%128, (ic4, s4, o32)]  bf16   (t=0: uniform 1/32)
  y    = cT.T @ x          -> ps_y  [(s,o)128, h256]      (16 MM ap256)
  yT   via 2 PE transposes -> yt_sb [h%128, (hc2, so128)] bf16
  vfT  = W @ yT            -> ps_vf [of%128, (g4, so128)] (8 MM ap128)
  diag: mask-mul (gpsimd) + free-reduce over o' (DVE) -> vr [of%128,(g4,s4)] f32
  mag  = I16.T @ vr^2 (PE) -> [o-sub 8, (g,s)16]; factor = exp(.5 ln m)/(1+m)
         (Ln+Exp share one act-table set -> no table reloads)
  fac128 = E8.T @ factor (PE broadcast over f partitions)
  vsq  = vr * fac128 -> bf16
  VmatT: vsq bcast over o' * mask (gpsimd) -> vp [of%128, (g4,s4,o'32)] bf16
  zT   = WT @ VmatT        -> ps_zt [h%128, (hc2, so128)] (8 MM ap128)
  b    = zt.T @ xT         -> ps_b  [(s,o)128, i512]      (8 MM ap512)
  eb   = exp(b) (Act, bf16); ebT via 4 PE transposes (bf16 PSUM)
  softmax over o in [i, (ic,s,o)] layout -> cT for next iter
Last iter stops after vsq; output transposed on PE and cast bf16->f32
by a gpsimd DMA.
"""

import numpy as np
import ml_dtypes

import concourse.bass as bass
import concourse.tile as tile
from concourse import mybir
from concourse.bass_utils import run_bass_kernel_spmd

F32 = mybir.dt.float32
BF16 = mybir.dt.bfloat16
AF = mybir.ActivationFunctionType
AX = mybir.AxisListType

B, I, H = 128, 512, 256
O, F = 32, 16
OF = O * F            # 512
NCORES = 8
S = B // NCORES       # 16 samples per core
Q = 4                 # quads per core
QS = 4                # samples per quad
NIT = 3
P = 128

# bf16 constant blob [128, CSTN]
CW = 0                # W  [h%128, (hc2, of512)]
CWT = CW + 2 * OF     # WT [of%128, (m4, h256)]
CID = CWT + 4 * H     # identity [128, 128]
CMD = CID + P         # diag mask [128, (g4, o32)]: md[p,(g,o)] = (o == g*8+p//16)
CI16 = CMD + 4 * O    # [128, 8]: i16[p, j] = (j == p//16)
CE8 = CI16 + 8        # [8, 128]: e8[j, p] = (p//16 == j)
CC0 = CE8 + P         # [128, 32] = 1/32
CIDF = CC0 + O        # 2 bf16 cols whose bytes alias to f32 1.0
CSTN = CIDF + 2


def ap(t, dims, off=0):
    """AP over tile/handle `t`: keep partition dim, explicit free dims."""
    a = t if isinstance(t, bass.AP) else t[:]
    return bass.AP(tensor=a.tensor, offset=a.offset + off,
                   ap=[list(a.ap[0])] + [list(d) for d in dims])


def f32view(a, off=0):
    """1x1 f32 alias of an SBUF AP's base (for sync-anchor reads only)."""
    t = a.tensor
    t2 = t if t.dtype == F32 else bass.SBTensorHandle(
        name=t.name, shape=[t.shape[0], t.shape[1] // 2], dtype=F32,
        base_partition=t.base_partition,
        manual_sbuf_range=t.manual_sbuf_range,
        manual_base_name=t.manual_base_name)
    return bass.AP(tensor=t2, offset=off,
                   ap=[[int(t2.shape[1]), 1], [1, 1]])


def dram_ap(handle, dims, off=0):
    """AP over DRAM handle with fully explicit dims (first = partition)."""
    a = handle[:]
    return bass.AP(tensor=a.tensor, offset=a.offset + off,
                   ap=[list(d) for d in dims])


MM_LABELS = []


def build_program(split_waits=True):
    MM_LABELS.clear()
    nc = bass.Bass("TRN2", target_bir_lowering=False)

    x_d = nc.dram_tensor("xb", [Q, P, QS * 4 * H], BF16, kind="ExternalInput")
    xt_d = nc.dram_tensor("xtb", [Q, P, QS * 2 * I], BF16, kind="ExternalInput")
    cst_d = nc.dram_tensor("cst", [P, CSTN], BF16, kind="ExternalInput")
    out_d = nc.dram_tensor("out", [S, OF], F32, kind="ExternalOutput")

    with tile.TileContext(nc) as tc:
        with (
            tc.tile_pool(name="consts", bufs=1) as consts,
            tc.tile_pool(name="xpool", bufs=4) as xpool,
            tc.tile_pool(name="xtpool", bufs=4) as xtpool,
            tc.tile_pool(name="work", bufs=2) as work,
            tc.tile_pool(name="ps", bufs=1, space="PSUM") as ps,
        ):
            cst = consts.tile([P, CSTN], BF16)
            nc.sync.dma_start(out=cst[:], in_=cst_d[:])
            w_sb = cst[:, CW:CW + 2 * OF]
            wt_sb = cst[:, CWT:CWT + 4 * H]
            id_sb = cst[:, CID:CID + P]
            md_sb = cst[:, CMD:CMD + 4 * O]
            i16_sb = cst[:, CI16:CI16 + 8]
            e8_sb = cst[:8, CE8:CE8 + P]
            c0_sb = cst[:, CC0:CC0 + O]

            # PE sync anchors: PE observes foreign engine clocks via 1x1
            # transposes so walrus can elide per-instruction waits.
            anch = None  # anchors disabled; bank freed for the vo tile
            idf1 = f32view(cst[:], off=CIDF // 2)
            dirty = {}
            acol = [0]
            pending = []

            def mark(key, apv):
                dirty[key] = apv

            def pe_sync(*keys, force=False):
                # Anchors proved to over-serialize (PE waits on the globally
                # newest op of an engine, not the actual dependency); rely on
                # Tile's precise per-instruction deps instead.
                pending.clear()
                if not force:
                    return
                for k in keys:
                    if k not in dirty:
                        continue
                    d = dirty.pop(k)
                    MM_LABELS.append("anch:" + k)
                    a = nc.tensor.transpose(
                        anch[:1, acol[0]:acol[0] + 1], f32view(d),
                        idf1)
                    pending.append(a.ins)
                    acol[0] = (acol[0] + 1) % 16

            def _chain(b):
                for a in pending:
                    bass._add_dep_helper(b.ins, a, sync=False,
                                         reason="pe-anchor order")
                return b

            def T(out, in_, ident, label="", **kw):
                MM_LABELS.append(label or CUR[0] + ":T")
                return _chain(nc.tensor.transpose(out, in_, ident, **kw))

            def MM(out, lhsT, rhs, label="", **kw):
                MM_LABELS.append(label or CUR[0])
                return _chain(nc.tensor.matmul(out, lhsT, rhs, **kw))

            mark("cst", cst)
            CUR = ["init"]

            # ---- input DMAs (x natural + pre-transposed, interleaved so
            # quad q's xT lands before its first b-stage) ----
            x_sb = {}
            xt_sb = {}

            NXW = QS * 4 * H

            def load_x(q):
                xs = xpool.tile([P, NXW], BF16, tag="x")
                nc.sync.dma_start(
                    out=ap(xs, [[1, NXW]]),
                    in_=dram_ap(x_d, [[NXW, P], [1, NXW]], off=q * P * NXW),
                )
                mark("x%d" % q, xs)
                x_sb[q] = xs

            def load_xt(q):
                xt = xtpool.tile([P, NXW], BF16, tag="xt")
                nc.sync.dma_start(
                    out=ap(xt, [[1, NXW]]),
                    in_=dram_ap(xt_d, [[NXW, P], [1, NXW]], off=q * P * NXW),
                )
                mark("xt%d" % q, xt)
                xt_sb[q] = xt

            for q in range(Q):
                load_x(q)
                load_xt(q)

            cT = {q: None for q in range(Q)}
            fin = {}

            def mk_chunks(q):
                """Per-quad list of emission chunks; wavefront-interleaved
                across quads so PE always has another quad's work during
                cross-engine chain latencies."""
                st = {}
                chunks = []
                for t in range(NIT):
                    chunks.extend(stage_fns(q, t, st))
                return chunks

            def stage_fns(q, t, st):
                last = (t == NIT - 1)

                def A():
                    CUR[0] = "A.q%d.t%d" % (q, t)
                    pe_sync("cst", "x%d" % q, "dve")
                    ps_y = ps.tile([P, H], F32, tag="y")
                    for s in range(QS):
                        for ic in range(4):
                            lhsT = (c0_sb if t == 0 else
                                    ap(cT[q], [[1, O]], off=ic * P + s * O))
                            MM(
                                ps_y[32 * s:32 * s + 32, :],
                                lhsT,
                                ap(x_sb[q], [[1, H]], off=s * 4 * H + ic * H),
                                start=(ic == 0),
                                stop=(ic == 3),
                                tile_position=(0, 32 * s),
                            )
                    ysb = work.tile([P, H], BF16, tag="ysb", bufs=4)
                    nc.scalar.activation(ysb[:], ps_y[:], AF.Copy)
                    mark("act", ysb)
                    st["ysb"] = ysb

                def Bc():
                    CUR[0] = "Bc.q%d.t%d" % (q, t)
                    pe_sync("act")
                    ps_yt = ps.tile([P, 4 * P], BF16, tag="ytb")
                    for hc in range(2):
                        T(ps_yt[:, hc * P:(hc + 1) * P],
                          st["ysb"][:, hc * P:(hc + 1) * P], id_sb)
                    ytsb = work.tile([P, 2 * P], BF16, tag="ytsb", bufs=4)
                    nc.scalar.activation(ytsb[:], ps_yt[:, :2 * P], AF.Copy)
                    mark("act", ytsb)
                    st["ytsb"] = ytsb

                def C():
                    CUR[0] = "C.q%d.t%d" % (q, t)
                    pe_sync("dve")
                    ps_vf = ps.tile([P, 4 * P], F32, tag="vf", bufs=2)
                    for g in range(4):
                        for hc in range(2):
                            MM(
                                ps_vf[:, g * P:(g + 1) * P],
                                ap(w_sb, [[1, P]], off=hc * OF + g * P),
                                ap(st["ytsb"], [[1, P]], off=hc * P),
                                start=(hc == 0),
                                stop=(hc == 1),
                            )
                    msk = work.tile([P, 4 * P], BF16, tag="msk")
                    nc.vector.tensor_mul(
                        ap(msk, [[P, 4], [O, QS], [1, O]]),
                        ap(ps_vf, [[P, 4], [O, QS], [1, O]]),
                        ap(md_sb, [[O, 4], [0, QS], [1, O]]),
                    )
                    vr = work.tile([P, 16], F32, tag="vr", bufs=4)
                    nc.vector.reduce_sum(
                        out=vr[:],
                        in_=ap(msk, [[O, 16], [1, O]]),
                        axis=AX.X,
                    )
                    vrsq = work.tile([P, 16], BF16, tag="vrsq", bufs=4)
                    nc.vector.tensor_mul(vrsq[:], vr[:], vr[:])
                    mark("dve", vrsq)
                    st["vr"] = vr
                    st["vrsq"] = vrsq

                def DE():
                    CUR[0] = "DE.q%d.t%d" % (q, t)
                    pe_sync("dve")
                    mf = ps.tile([P, 32], F32, tag="mf")
                    MM(mf[:8, :16], i16_sb, st["vrsq"][:])
                    lnm = work.tile([8, 16], F32, tag="lnm")
                    nc.scalar.activation(lnm[:], mf[:8, :16], AF.Ln)
                    s0 = work.tile([8, 16], F32, tag="s0")
                    nc.scalar.activation(s0[:], lnm[:], AF.Exp, scale=0.5)
                    mark("act", s0)
                    onep = work.tile([8, 16], F32, tag="onep")
                    nc.vector.tensor_scalar_add(onep[:], mf[:8, :16], 1.0)
                    rp = work.tile([8, 16], F32, tag="rp")
                    nc.vector.reciprocal(rp[:], onep[:])
                    facb = work.tile([8, 16], BF16, tag="facb", bufs=4)
                    nc.vector.tensor_mul(facb[:], s0[:], rp[:])
                    mark("dve", facb)
                    pe_sync("dve", "act")
                    MM(mf[:, 16:32], e8_sb, facb[:8, :])
                    if not last:
                        vsq = work.tile([P, 16], BF16, tag="vsq", bufs=4)
                        nc.vector.tensor_mul(vsq[:], st["vr"][:],
                                             mf[:, 16:32])
                        vp = work.tile([P, 4 * P], BF16, tag="vp", bufs=4)
                        nc.gpsimd.tensor_mul(
                            ap(vp, [[P, 4], [O, QS], [1, O]]),
                            ap(vsq, [[4, 4], [1, QS], [0, O]]),
                            ap(md_sb, [[O, 4], [0, QS], [1, O]]),
                        )
                        mark("pool", vp)
                        st["vp"] = vp
                    else:
                        # all quads write one [128, (q, s, g)] tile; single
                        # transpose + copy + cast-DMA at the last quad
                        if "vsq_all" not in fin:
                            fin["vsq_all"] = work.tile([P, 64], BF16,
                                                       tag="vsq_all", bufs=1,
                                                       name="vsq_all")
                        nc.vector.tensor_mul(
                            ap(fin["vsq_all"], [[1, 4], [4, 4]], off=16 * q),
                            ap(st["vr"], [[4, 4], [1, 4]]),
                            ap(mf, [[4, 4], [1, 4]], off=16),
                        )
                        fin["done"] = fin.get("done", 0) + 1
                        if fin["done"] == Q:
                            ps_vo = ps.tile([P, P], BF16, tag="vo",
                                            name="ps_vo")
                            T(ps_vo[:64, :P], fin["vsq_all"][:], id_sb)
                            vo = work.tile([64, P], BF16, tag="vosb")
                            nc.vector.tensor_copy(vo[:], ps_vo[:64, :P])
                            nc.gpsimd.dma_start(
                                out=dram_ap(out_d, [[P, 64], [1, P]]),
                                in_=vo[:],
                            )

                def Fc():
                    CUR[0] = "Fc.q%d.t%d" % (q, t)
                    pe_sync("dve")
                    ps_zt = ps.tile([P, 2 * P], F32, tag="zt")
                    for hc in range(2):
                        for m in range(4):
                            MM(
                                ps_zt[:, hc * P:(hc + 1) * P],
                                ap(wt_sb, [[1, P]], off=m * H + hc * P),
                                ap(st["vp"], [[1, P]], off=m * P),
                                start=(m == 0),
                                stop=(m == 3),
                            )
                    ztsb = work.tile([P, 2 * P], BF16, tag="ztsb", bufs=4)
                    nc.scalar.activation(ztsb[:], ps_zt[:], AF.Copy)
                    mark("act", ztsb)
                    st["ztsb"] = ztsb

                def G():
                    CUR[0] = "G.q%d.t%d" % (q, t)
                    pe_sync("act", "xt%d" % q)
                    ps_b = ps.tile([P, I], F32, tag="b")
                    for s in range(QS):
                        for hc in range(2):
                            MM(
                                ps_b[32 * s:32 * s + 32, :],
                                ap(st["ztsb"], [[1, O]], off=hc * P + s * O),
                                ap(xt_sb[q], [[1, I]], off=s * 2 * I + hc * I),
                                start=(hc == 0),
                                stop=(hc == 1),
                                tile_position=(0, 32 * s),
                            )
                    eb = work.tile([P, I], BF16, tag="eb", bufs=4)
                    nc.scalar.activation(eb[:], ps_b[:], AF.Exp)
                    mark("act", eb)
                    st["eb"] = eb

                def Hc():
                    CUR[0] = "Hc.q%d.t%d" % (q, t)
                    pe_sync("act")
                    ps_ebt = ps.tile([P, 4 * P], BF16, tag="ytb")
                    for ic in range(4):
                        T(ps_ebt[:, ic * P:(ic + 1) * P],
                          st["eb"][:, ic * P:(ic + 1) * P], id_sb)
                    ssum = work.tile([P, 16], F32, tag="ssum")
                    nc.vector.reduce_sum(
                        out=ssum[:],
                        in_=ap(ps_ebt, [[O, 16], [1, O]]),
                        axis=AX.X,
                    )
                    rs = work.tile([P, 16], F32, tag="rs")
                    nc.vector.reciprocal(rs[:], ssum[:])
                    ct = work.tile([P, 4 * P], BF16, tag="ct%d" % q, bufs=1)
                    nc.vector.tensor_mul(
                        ap(ct, [[O, 16], [1, O]]),
                        ap(ps_ebt, [[O, 16], [1, O]]),
                        ap(rs, [[1, 16], [0, O]]),
                    )
                    mark("dve", ct)
                    cT[q] = ct

                if last:
                    return [A, Bc, C, DE]
                return [A, Bc, C, DE, Fc, G, Hc]

            all_chunks = {q: mk_chunks(q) for q in range(Q)}
            L = len(all_chunks[0])
            for k in range(L + Q - 1):
                for q in reversed(range(Q)):
                    c = k - q
                    if 0 <= c < L:
                        all_chunks[q][c]()

            if False:
                pass

                # ======== stage A: y, then yT ========
                y_sbq = {}
                yt_sbq = {}
                for q in range(Q):
                    pe_sync("cst", "x%d" % q, "dve")
                    ps_y = ps.tile([P, H], F32, tag="y")
                    for s in range(QS):
                        for ic in range(4):
                            lhsT = (c0_sb if t == 0 else
                                    ap(cT[q], [[1, O]], off=ic * P + s * O))
                            MM(
                                ps_y[32 * s:32 * s + 32, :],
                                lhsT,
                                ap(x_sb[q], [[1, H]], off=s * 4 * H + ic * H),
                                start=(ic == 0),
                                stop=(ic == 3),
                                tile_position=(0, 32 * s),
                            )
                    ysb = work.tile([P, H], BF16, tag="ysb", bufs=4)
                    nc.scalar.activation(ysb[:], ps_y[:], AF.Copy)
                    mark("act", ysb)
                    y_sbq[q] = ysb
                for q in range(Q):
                    pe_sync("act")
                    ps_yt = ps.tile([P, 4 * P], BF16, tag="ytb")
                    for hc in range(2):
                        T(ps_yt[:, hc * P:(hc + 1) * P],
                          y_sbq[q][:, hc * P:(hc + 1) * P], id_sb)
                    ytsb = work.tile([P, 2 * P], BF16, tag="ytsb", bufs=4)
                    nc.vector.tensor_copy(ytsb[:], ps_yt[:, :2 * P])
                    mark("dve", ytsb)
                    yt_sbq[q] = ytsb

                # ======== stage B (sw-pipelined by one quad) ========
                # per q: vfT (PE) -> msk (gpsimd) -> vr,vrsq (DVE)
                #        -> mag (PE) -> ln/exp (Act) -> 1+m,recip,fac (DVE)
                #        -> [q-1: fac128 (PE) -> vsq (DVE) -> vp (gpsimd)]
                vr_q = {}
                mf_q = {}
                fac_q = {}
                vp_q = {}

                def emit_tail(qq):
                    # fac128 broadcast, vsq; VmatT only when not last iter
                    pe_sync("dve", "act")
                    MM(mf_q[qq][:, 16:32], e8_sb, fac_q[qq][:8, :])
                    if not last:
                        vsq = work.tile([P, 16], BF16, tag="vsq", bufs=4)
                        nc.vector.tensor_mul(vsq[:], vr_q[qq][:],
                                             mf_q[qq][:, 16:32])
                        mark("dve", vsq)
                        vp = work.tile([P, 4 * P], BF16, tag="vp", bufs=4)
                        nc.gpsimd.tensor_mul(
                            ap(vp, [[P, 4], [O, QS], [1, O]]),
                            ap(vsq, [[4, 4], [1, QS], [0, O]]),
                            ap(md_sb, [[O, 4], [0, QS], [1, O]]),
                        )
                        mark("pool", vp)
                        vp_q[qq] = vp
                    else:
                        # (s,g) free order so the PE transpose lands the
                        # output with a single uniform DRAM stride
                        vsq = work.tile([P, 16], BF16, tag="vsq", bufs=4)
                        nc.vector.tensor_mul(
                            ap(vsq, [[1, 4], [4, 4]]),
                            ap(vr_q[qq], [[4, 4], [1, 4]]),
                            ap(mf_q[qq], [[4, 4], [1, 4]], off=16),
                        )
                        mark("dve", vsq)
                        pe_sync("dve")
                        ps_vo = ps.tile([P, 4 * P], BF16, tag="ytb")
                        T(ps_vo[:16, :P], vsq[:], id_sb)
                        vo = work.tile([16, P], BF16, tag="vo")
                        nc.vector.tensor_copy(vo[:], ps_vo[:16, :P])
                        nc.gpsimd.dma_start(
                            out=dram_ap(out_d, [[P, 16], [1, P]],
                                        off=qq * QS * OF),
                            in_=vo[:],
                        )

                for q in range(Q):
                    pe_sync("dve")
                    ps_vf = ps.tile([P, 4 * P], F32, tag="vf", bufs=2)
                    for g in range(4):
                        for hc in range(2):
                            MM(
                                ps_vf[:, g * P:(g + 1) * P],
                                ap(w_sb, [[1, P]], off=hc * OF + g * P),
                                ap(yt_sbq[q], [[1, P]], off=hc * P),
                                start=(hc == 0),
                                stop=(hc == 1),
                            )
                    msk = work.tile([P, 4 * P], BF16, tag="msk")
                    nc.vector.tensor_mul(
                        ap(msk, [[P, 4], [O, QS], [1, O]]),
                        ap(ps_vf, [[P, 4], [O, QS], [1, O]]),
                        ap(md_sb, [[O, 4], [0, QS], [1, O]]),
                    )
                    mark("dve", msk)
                    vr = work.tile([P, 16], F32, tag="vr", bufs=4)
                    nc.vector.reduce_sum(
                        out=vr[:],
                        in_=ap(msk, [[O, 16], [1, O]]),
                        axis=AX.X,
                    )
                    vrsq = work.tile([P, 16], BF16, tag="vrsq", bufs=4)
                    nc.vector.tensor_mul(vrsq[:], vr[:], vr[:])
                    mark("dve", vrsq)
                    vr_q[q] = vr

                    if q > 0:
                        emit_tail(q - 1)

                    pe_sync("dve", "pool")
                    mf = ps.tile([P, 32], F32, tag="mf")
                    MM(mf[:8, :16], i16_sb, vrsq[:])
                    mf_q[q] = mf

                    lnm = work.tile([8, 16], F32, tag="lnm")
                    nc.scalar.activation(lnm[:], mf[:8, :16], AF.Ln)
                    s0 = work.tile([8, 16], F32, tag="s0")
                    nc.scalar.activation(s0[:], lnm[:], AF.Exp, scale=0.5)
                    mark("act", s0)
                    onep = work.tile([8, 16], F32, tag="onep")
                    nc.vector.tensor_scalar_add(onep[:], mf[:8, :16], 1.0)
                    rp = work.tile([8, 16], F32, tag="rp")
                    nc.vector.reciprocal(rp[:], onep[:])
                    facb = work.tile([8, 16], BF16, tag="facb", bufs=4)
                    nc.vector.tensor_mul(facb[:], s0[:], rp[:])
                    mark("dve", facb)
                    fac_q[q] = facb
                emit_tail(Q - 1)

                if last:
                    continue

                # ======== zT + PSUM->SBUF copy ========
                zt_sbq = {}
                for q in range(Q):
                    pe_sync("pool", "dve")
                    ps_zt = ps.tile([P, 2 * P], F32, tag="zt")
                    for hc in range(2):
                        for m in range(4):
                            MM(
                                ps_zt[:, hc * P:(hc + 1) * P],
                                ap(wt_sb, [[1, P]], off=m * H + hc * P),
                                ap(vp_q[q], [[1, P]], off=m * P),
                                start=(m == 0),
                                stop=(m == 3),
                            )
                    ztsb = work.tile([P, 2 * P], BF16, tag="ztsb", bufs=4)
                    nc.scalar.activation(ztsb[:], ps_zt[:], AF.Copy)
                    mark("act", ztsb)
                    zt_sbq[q] = ztsb

                # ======== stage C: b -> exp -> ebT -> softmax ========
                eb_sbq = {}
                for q in range(Q):
                    pe_sync("act", "xt%d" % q)
                    ps_b = ps.tile([P, I], F32, tag="b")
                    for s in range(QS):
                        for hc in range(2):
                            MM(
                                ps_b[32 * s:32 * s + 32, :],
                                ap(zt_sbq[q], [[1, O]], off=hc * P + s * O),
                                ap(xt_sb[q], [[1, I]], off=s * 2 * I + hc * I),
                                start=(hc == 0),
                                stop=(hc == 1),
                                tile_position=(0, 32 * s),
                            )
                    eb = work.tile([P, I], BF16, tag="eb", bufs=4)
                    nc.scalar.activation(eb[:], ps_b[:], AF.Exp)
                    mark("act", eb)
                    eb_sbq[q] = eb
                for q in range(Q):
                    pe_sync("act")
                    ps_ebt = ps.tile([P, 4 * P], BF16, tag="ytb")
                    for ic in range(4):
                        T(ps_ebt[:, ic * P:(ic + 1) * P],
                          eb_sbq[q][:, ic * P:(ic + 1) * P], id_sb)
                    ssum = work.tile([P, 16], F32, tag="ssum")
                    nc.vector.reduce_sum(
                        out=ssum[:],
                        in_=ap(ps_ebt, [[O, 16], [1, O]]),
                        axis=AX.X,
                    )
                    rs = work.tile([P, 16], F32, tag="rs")
                    nc.vector.reciprocal(rs[:], ssum[:])
                    ct = work.tile([P, 4 * P], BF16, tag="ct%d" % q, bufs=1)
                    nc.vector.tensor_mul(
                        ap(ct, [[O, 16], [1, O]]),
                        ap(ps_ebt, [[O, 16], [1, O]]),
                        ap(rs, [[1, 16], [0, O]]),
                    )
                    mark("dve", ct)
                    cT[q] = ct

    if split_waits:
        _split_fat_waits(nc)
    return nc


def _split_fat_waits(nc, maxw=1):
    """Walrus caps sync waits per instruction; split overflow onto extra
    same-engine Drain instructions inserted just before the offender."""
    nsplit = 0
    for blk in nc.m.functions[0].blocks:
        new_insts = []
        for inst in blk.instructions:
            si = getattr(inst, "sync_info", None)
            w = list(si.on_wait) if si is not None and si.on_wait else []
            if len(w) > maxw:
                for k in range(0, len(w) - maxw, maxw):
                    d = mybir.InstDrain(name="I-waitsplit-%d" % nsplit,
                                        ins=[], outs=[])
                    nsplit += 1
                    d.engine = inst.engine
                    d.sync_info = mybir.SyncInfo(on_wait=w[k:k + maxw],
                                                 on_update=[])
                    new_insts.append(d)
                si.on_wait = w[len(w) - maxw:]
            new_insts.append(inst)
        blk.instructions[:] = new_insts
    return nc


_NC_CACHE = None


def make_cst(Wn):
    """bf16 constant blob [128, CSTN] matching the device-side layout."""
    cst = np.zeros((P, CSTN), np.float32)
    cst[:, CW:CW + 2 * OF] = (
        Wn.reshape(2, P, OF).transpose(1, 0, 2).reshape(P, 2 * OF))
    cst[:, CWT:CWT + 4 * H] = (
        Wn.T.reshape(4, P, H).transpose(1, 0, 2).reshape(P, 4 * H))
    cst[:, CID:CID + P] = np.eye(P, dtype=np.float32)
    for p in range(P):
        for g in range(4):
            cst[p, CMD + g * O + g * 8 + p // 16] = 1.0
    cst[np.arange(P), CI16 + np.arange(P) // 16] = 1.0
    for j in range(8):
        cst[j, CE8 + 16 * j:CE8 + 16 * (j + 1)] = 1.0
    cst[:, CC0:CC0 + O] = 1.0 / O
    out = cst.astype(ml_dtypes.bfloat16)
    # bf16 pair (0.0, 1.0) little-endian == f32 1.0 when viewed 4-byte
    out[:, CIDF] = ml_dtypes.bfloat16(0.0)
    out[:, CIDF + 1] = ml_dtypes.bfloat16(1.0)
    return out


def make_in_maps(x, W):
    x = np.asarray(x, dtype=np.float32)
    Wn = np.asarray(W, dtype=np.float32).reshape(H, OF)
    cst = make_cst(Wn)
    xq = x.astype(ml_dtypes.bfloat16).reshape(NCORES, Q, QS, 4, P, H)
    # xb[c, q, p, (s, ic, h)] = x[c, 4q+s, 128ic+p, h]
    xb = np.ascontiguousarray(xq.transpose(0, 1, 4, 2, 3, 5)).reshape(
        NCORES, Q, P, QS * 4 * H)
    # xtb[c, q, p, (s, hc, i)] = x[c, 4q+s, i, 128hc+p]
    xth = x.astype(ml_dtypes.bfloat16).reshape(NCORES, Q, QS, I, 2, P)
    xtb = np.ascontiguousarray(xth.transpose(0, 1, 5, 2, 4, 3)).reshape(
        NCORES, Q, P, QS * 2 * I)
    return [
        {"xb": xb[c], "xtb": xtb[c], "cst": cst}
        for c in range(NCORES)
    ]


def kernel(x: np.ndarray, W: np.ndarray) -> np.ndarray:
    global _NC_CACHE
    if _NC_CACHE is None:
        _NC_CACHE = build_program()
    in_maps = make_in_maps(x, W)
    res = run_bass_kernel_spmd(_NC_CACHE, in_maps, core_ids=list(range(NCORES)))
    out = np.stack([res.results[c]["out"] for c in range(NCORES)])
    return out.reshape(B, O, F)
